# revision 42
# baseline (speedup 1.0000x reference)
"""Trainium2 Bass kernel for GQA attention (B=1,T=2048,D=3584,N=28,KH=4,H=128).

Sharding: 8 cores, one KV head per core-pair. Core c handles kv head c//2 and
query heads [7*(c//2)+4*(c%2) : ...] (4 on even cores, 3+1 dummy on odd cores,
padded to 4 so all cores run one SPMD program).

Per-core dataflow (all "T"-suffixed tensors are feature-major, i.e. transposed):
  stage 1: xT[D,T] (bf16) @ wqk (bf16) -> 6 PSUM units [128,512] per t-stripe;
    each unit drains through an ACT Identity+bias copy (frees the PSUM bank
    fast), then RoPE (plain mul/add) on DVE -> qkT/kT [H,T] f32. The last
    stripe's RoPE is deferred into stage 2 (only window 3 reads it) so the
    DVE queue is empty at the stage boundary. v: bias via ACT -> vT bf16 ->
    v_nat via DMA transpose on the sync queue (no PE, no ACT occupancy).
  stage 2 (causal suffix tiles): per (head, 512-wide t-window) loop over key
    chunks sc: scoresT[s, toff..] = kT_chunk^T . qT_suffix (fp32r, one
    matmul per chunk covering all valid t of the window) -> exp on ACT
    (bf16) -> triangular mask on the 128 diagonal cols (DVE) -> denominator
    accumulated in SBUF (DVE adds) + one ones-matmul per (head,window) ->
    attnV accumulates out_ps[128,512] over chunks -> normalize with DVE
    reciprocal*mul -> attnT bf16. Windows processed in order [1,2,3,0].
  stage 3 (transposed): yT[d_block,t] = sum_u woT_u_block^T . attnT_u
    (bf16) -> bf16 partial yT, summed + transposed on host. Stage-3 groups
    of each finished window are interleaved into the next window's score/
    attnV stream as PE filler, so exp (ACT) latency never starves the PE.
"""

import numpy as np
import ml_dtypes
from contextlib import ExitStack

import concourse.bass as bass
import concourse.bacc as bacc
import concourse.tile as tile
from concourse import mybir
from concourse.bass_utils import run_bass_kernel_spmd

F32 = mybir.dt.float32
F32R = mybir.dt.float32r
BF16 = mybir.dt.bfloat16

B, T, D = 1, 2048, 3584
N, KH, H = 28, 4, 128
G = N // KH              # 7 query heads per kv head
NQ = 4                   # padded query heads per core
NU = NQ + 2              # q0..q3, k, v projection units
DC = D // 128            # 28 contraction chunks
TS = 512                 # stage-1 moving-dim tile
NT = T // TS             # 4
SCC = T // 128           # 16 key chunks
W = 512                  # stage-2/3 t-window (matmul moving dim <= 512)
NWIN = T // W            # 4 windows
SCALE = float(H) ** -0.5
ADD = mybir.AluOpType.add
MUL = mybir.AluOpType.mult

_TRACE = False           # test.py flips this to get an NTFF profile


def build_program(sched):
    """sched: per t-half list of (sc, toff, L, mode); mode 0=clean, 1=diag."""
    nc = bacc.Bacc(None)
    _build_body(nc, sched)
    nc.compile()
    return nc


def _build_body(nc, sched):
    xT_d = nc.dram_tensor("xT", [D, T], BF16, kind="ExternalInput")
    wqk_d = nc.dram_tensor("wqk", [DC, 128, NU * 128], BF16, kind="ExternalInput")
    bias_d = nc.dram_tensor("biasT", [128, NU], F32, kind="ExternalInput")
    cos_d = nc.dram_tensor("cosT", [128, T], F32, kind="ExternalInput")
    sin_d = nc.dram_tensor("sinT", [128, T], F32, kind="ExternalInput")
    triu_d = nc.dram_tensor("triu", [128, 128], BF16, kind="ExternalInput")
    ones_d = nc.dram_tensor("ones", [128, 128], BF16, kind="ExternalInput")
    wo_d = nc.dram_tensor("woT", [128, NQ, D], BF16, kind="ExternalInput")
    yT_d = nc.dram_tensor("yT", [D, T], BF16, kind="ExternalOutput")

    with tile.TileContext(nc) as tc, ExitStack() as ctx:
        persist = ctx.enter_context(tc.tile_pool(name="persist", bufs=1))

        qkT = [persist.tile([128, T], F32R, tag=f"qkT{u}", name=f"qkT{u}")
               for u in range(NQ + 1)]
        # biased (pre-RoPE) last-stripe projections; their RoPE is deferred
        # into stage 2 (results only needed by window 3) so the DVE queue
        # is clear at the stage-1/2 boundary.
        tail_tmp = [persist.tile([128, TS], F32, tag=f"tail{u}",
                                 name=f"tail{u}")
                    for u in range(NQ + 1)]
        v_nat = persist.tile([128, SCC, 128], BF16, tag="vnat")
        attnT = [persist.tile([128, T], BF16, tag=f"attnT{u}", name=f"attnT{u}")
                 for u in range(NQ)]
        cos_sb = persist.tile([128, T], F32, tag="cos")
        sin_sb = persist.tile([128, T], F32, tag="sin")
        bias_sb = persist.tile([128, NU], F32, tag="bias")
        triu_sb = persist.tile([128, 128], BF16, tag="triu")
        ones_sb = persist.tile([128, 128], BF16, tag="ones")

        # ---------------- stage 1: QKV projections + RoPE ----------------
        with tc.tile_pool(name="s1", bufs=1) as wpool, \
             tc.tile_pool(name="s1x", bufs=12) as xpool, \
             tc.tile_pool(name="s1s", bufs=2) as spool, \
             tc.tile_pool(name="s1t", bufs=2) as tpool, \
             tc.tile_pool(name="s1v", bufs=1) as vpool, \
             tc.tile_pool(name="s1ps", bufs=1, space="PSUM") as ps1:

            wqk_sb = [wpool.tile([128, NU * 128], BF16, tag=f"wqk{dc}",
                                 name=f"wqk{dc}") for dc in range(DC)]
            vT_sb = vpool.tile([128, T], BF16)

            for ts in range(NT):
                tsl = slice(ts * TS, (ts + 1) * TS)
                ps = [ps1.tile([128, TS], F32, tag=f"u{u}", name=f"ps_u{u}")
                      for u in range(NU)]
                def epilogue(u):
                    # Drain PSUM via a fast ACT copy that folds the bias
                    # (out = ps + bias) so the next stripe's matmuls (and
                    # stage 2) aren't gated on the slow DVE RoPE chain
                    # reading PSUM. Last stripe's q/k RoPE is deferred
                    # into stage 2 (only window 3 needs it).
                    if u == NU - 1:
                        # v: bias only, un-rotated, bf16; natural layout
                        # via DMA transpose on the sync queue (transpose
                        # triggers occupy their HWDGE engine >1us each and
                        # would delay stage-2 exps if put on ACT).
                        nc.scalar.activation(
                            vT_sb[:, tsl], ps[u][:],
                            mybir.ActivationFunctionType.Identity,
                            bias=bias_sb[:, u:u + 1])
                        for sc in range(ts * (TS // 128),
                                        (ts + 1) * (TS // 128)):
                            nc.sync.dma_start(
                                v_nat[:, sc, :],
                                vT_sb[:, sc * 128:(sc + 1) * 128],
                                transpose=True)
                        return
                    if ts == NT - 1:
                        nc.scalar.activation(
                            tail_tmp[u][:], ps[u][:],
                            mybir.ActivationFunctionType.Identity,
                            bias=bias_sb[:, u:u + 1])
                        return
                    tmp = tpool.tile([128, TS], F32, tag="s1tmp", bufs=5)
                    nc.scalar.activation(
                        tmp[:], ps[u][:],
                        mybir.ActivationFunctionType.Identity,
                        bias=bias_sb[:, u:u + 1])
                    src = tmp
                    dst = qkT[u][:, tsl]
                    c2, s2 = cos_sb[:, tsl], sin_sb[:, tsl]
                    scrA = spool.tile([128, TS], F32, tag="ropeA")
                    scrB = spool.tile([128, TS], F32, tag="ropeB")
                    nc.vector.tensor_mul(scrA[0:64, :], src[64:128, :],
                                         s2[64:128, :])
                    nc.vector.tensor_mul(scrA[64:128, :], src[0:64, :],
                                         s2[0:64, :])
                    nc.vector.tensor_mul(scrB[:], src[:], c2)
                    nc.vector.tensor_add(dst, scrB[:], scrA[:])

                for dc in range(DC):
                    if ts == 0:
                        # interleave weight loads with the first activation
                        # stripe so the PE starts early; misc loads go on
                        # the scalar DMA queue so they never delay x chunks.
                        # weights split across the scalar HWDGE and gpsimd
                        # SWDGE queues: one SWDGE stream alone generates at
                        # ~1.04us/chunk while the PE consumes a chunk every
                        # ~1.3us — no margin. Two queues give 2x headroom.
                        weng = nc.scalar if (dc < 2 or dc % 2) else nc.gpsimd
                        weng.dma_start(wqk_sb[dc][:], wqk_d[dc])
                        if dc == 2:
                            nc.scalar.dma_start(bias_sb[:], bias_d[:])
                            nc.scalar.dma_start(triu_sb[:], triu_d[:])
                            nc.scalar.dma_start(ones_sb[:], ones_d[:])
                        elif dc == 16:
                            # 2MB of rope tables deferred past the startup
                            # DMA crunch (first use is the stripe epilogue)
                            nc.scalar.dma_start(cos_sb[:], cos_d[:])
                            nc.scalar.dma_start(sin_sb[:], sin_d[:])
                    xt = xpool.tile([128, TS], BF16)
                    nc.sync.dma_start(
                        xt[:], xT_d[dc * 128:(dc + 1) * 128, tsl])
                    for u in range(NU):
                        nc.tensor.matmul(
                            ps[u][:],
                            wqk_sb[dc][:, u * 128:(u + 1) * 128],
                            xt[:],
                            start=(dc == 0), stop=(dc == DC - 1))
                        if dc == DC - 1:
                            # emit each unit's drain right after its last
                            # matmul so ACT overlaps the remaining units
                            epilogue(u)

        # ---------------- stage 2 + 3 ----------------
        wopool = ctx.enter_context(tc.tile_pool(name="s3w", bufs=1))
        wo_sb = wopool.tile([128, NQ, D], BF16)
        for u in range(NQ):
            nc.sync.dma_start(wo_sb[:, u, :], wo_d[:, u, :])

        kT = qkT[NQ]

        with tc.tile_pool(name="s2pt", bufs=4) as ptpool, \
             tc.tile_pool(name="s2acc", bufs=2) as accpool, \
             tc.tile_pool(name="s2r", bufs=2) as rpool, \
             tc.tile_pool(name="s3y", bufs=4) as ysbpool, \
             tc.tile_pool(name="s2ps", bufs=1, space="PSUM") as ps2, \
             tc.tile_pool(name="s3ps", bufs=1, space="PSUM") as ps3:

            ncopy = [0]

            def emit_s3_group(db, qt):
                dsl = slice(db * 128, (db + 1) * 128)
                wsl = slice(qt * W, (qt + 1) * W)
                y_ps = ps3.tile([128, W], F32, tag="yT", bufs=2)
                for u in range(NQ):
                    nc.tensor.matmul(
                        y_ps[:], wo_sb[:, u, dsl], attnT[u][:, wsl],
                        start=(u == 0), stop=(u == NQ - 1))
                y_sb = ysbpool.tile([128, W], BF16, tag="ysb")
                # GPSIMD cannot read PSUM; alternate DVE/ACT for the drain
                if ncopy[0] % 2 == 0:
                    nc.vector.tensor_copy(y_sb[:], y_ps[:])
                else:
                    nc.scalar.copy(y_sb[:], y_ps[:])
                ncopy[0] += 1
                nc.sync.dma_start(yT_d[dsl, wsl], y_sb[:])

            def attn_head_window(hq, qt, filler):
                tiles = sched[qt]
                n = len(tiles)
                wsl = slice(qt * W, (qt + 1) * W)
                out_ps = ps2.tile([128, W], F32, tag="out", bufs=3)
                acc = accpool.tile([128, W], BF16, tag="acc")
                pend = None

                def consume(i, sc_ps):
                    sc, toff, L, mode = tiles[i]
                    pt = ptpool.tile([128, W], BF16, tag="pt")
                    nc.scalar.activation(
                        pt[:, 0:L], sc_ps[:, 0:L],
                        mybir.ActivationFunctionType.Exp, scale=SCALE)
                    if mode == 1:
                        nc.vector.tensor_mul(
                            pt[:, 0:128], pt[:, 0:128], triu_sb[:])
                    if i == 0:
                        nc.vector.tensor_copy(acc[:], pt[:])
                    else:
                        nc.vector.tensor_add(
                            acc[:, toff:W], acc[:, toff:W], pt[:, 0:L])
                    nc.tensor.matmul(
                        out_ps[:, toff:W], v_nat[:, sc, :], pt[:, 0:L],
                        start=(i == 0), stop=(i == n - 1),
                        skip_group_check=True)

                for i, (sc, toff, L, mode) in enumerate(tiles):
                    tstart = qt * W + toff
                    sc_ps = ps2.tile([128, W], F32, tag="sc", bufs=3)
                    nc.tensor.matmul(
                        sc_ps[:, 0:L],
                        kT[:, sc * 128:(sc + 1) * 128],
                        qkT[hq][:, tstart:(qt + 1) * W],
                        start=True, stop=True)
                    if filler:
                        filler(i)
                    if pend is not None:
                        consume(*pend)
                    pend = (i, sc_ps)
                consume(*pend)

                den_ps = ps2.tile([128, W], F32, tag="sc", bufs=3)
                nc.tensor.matmul(den_ps[:], ones_sb[:], acc[:],
                                 start=True, stop=True)
                recip = rpool.tile([128, W], F32, tag="recip")
                nc.vector.reciprocal_approx_fast(recip[:], den_ps[:])
                nc.vector.tensor_mul(attnT[hq][:, wsl], out_ps[:], recip[:])

            def emit_tail_rope(u):
                # deferred last-stripe RoPE (needed only by window 3)
                tsl = slice(T - TS, T)
                src = tail_tmp[u]
                dst = qkT[u][:, tsl]
                c2, s2 = cos_sb[:, tsl], sin_sb[:, tsl]
                scrA = rpool.tile([128, TS], F32, tag="dropeA")
                scrB = rpool.tile([128, TS], F32, tag="dropeB")
                nc.vector.tensor_mul(scrA[0:64, :], src[64:128, :],
                                     s2[64:128, :])
                nc.vector.tensor_mul(scrA[64:128, :], src[0:64, :],
                                     s2[0:64, :])
                nc.vector.tensor_mul(scrB[:], src[:], c2)
                nc.vector.tensor_add(dst, scrB[:], scrA[:])

            # Window order [1,2,3,0]: stage-3 groups of the previously
            # finished window interleave as PE filler between the score/
            # attnV matmuls, so exp latency (ACT) never starves the PE.
            # Window 0 (smallest, most latency-bound) runs last, with
            # window-3 fillers available. The deferred last-stripe RoPE
            # (kT after the first window, q units during the second) lands
            # well before window 3 needs it.
            s3q = []
            for wi, qt in enumerate([1, 2, 3, 0]):
                nslots = NQ * len(sched[qt])
                stride = max(1, nslots // 28)

                def filler(i, stride=stride):
                    if s3q and i % stride == 0:
                        emit_s3_group(*s3q.pop(0))

                for hq in range(NQ):
                    attn_head_window(hq, qt, filler)
                    if wi == 1:
                        emit_tail_rope(hq)  # one q unit per head slot
                if wi == 0:
                    emit_tail_rope(NQ)      # kT first
                s3q += [(db, qt) for db in range(DC)]

            # drain last window's stage-3 groups
            while s3q:
                emit_s3_group(*s3q.pop(0))


def _suffix_sched(m):
    """m: bool [T, S]. Per t-half, suffix tiles (sc, toff, L, mode)."""
    tri = np.tril(np.ones((128, 128), bool))
    scheds = []
    for hf in range(NWIN):
        t0 = hf * W
        tiles = []
        for sc in range(SCC):
            sub = m[t0:t0 + W, sc * 128:(sc + 1) * 128]
            rows = sub.any(axis=1)
            if not rows.any():
                continue
            tstart = int(np.argmax(rows))
            if not rows[tstart:].all():
                raise ValueError("mask is not a row-suffix per chunk")
            L = W - tstart
            region = sub[tstart:, :]
            if region.all():
                mode = 0
            elif (L >= 128 and (region[:128] == tri).all()
                  and region[128:].all()):
                mode = 1
            else:
                raise ValueError("unsupported mask pattern")
            tiles.append((sc, tstart, L, mode))
        if not tiles or tiles[0][1] != 0:
            raise ValueError("first chunk must cover the full window")
        scheds.append(tiles)
    return scheds


def kernel(x, attn_mask, sin, cos, wq, wk, wv, wo, q_bias, k_bias, v_bias):
    x = np.asarray(x, np.float32)
    mask = np.asarray(attn_mask).astype(bool)
    sin = np.asarray(sin, np.float32)
    cos = np.asarray(cos, np.float32)
    wq = np.asarray(wq, np.float32)
    wk = np.asarray(wk, np.float32)
    wv = np.asarray(wv, np.float32)
    wo = np.asarray(wo, np.float32)
    q_bias = np.asarray(q_bias, np.float32).reshape(N, H)
    k_bias = np.asarray(k_bias, np.float32).reshape(KH, H)
    v_bias = np.asarray(v_bias, np.float32).reshape(KH, H)

    sched = _suffix_sched(mask[0])

    xT = np.ascontiguousarray(x[0].T).astype(ml_dtypes.bfloat16)   # [D, T]
    c = cos[0].T                                            # [64, T]
    s = sin[0].T
    cosT = np.ascontiguousarray(np.concatenate([c, c], 0))  # [128, T]
    sinT = np.ascontiguousarray(np.concatenate([s, -s], 0))
    triu = np.triu(np.ones((128, 128), np.float32)).astype(ml_dtypes.bfloat16)
    ones128 = np.ones((128, 128), ml_dtypes.bfloat16)

    in_maps = []
    for cidx in range(8):
        kv = cidx // 2
        qh = list(range(7 * kv + 4 * (cidx % 2),
                        7 * kv + (4 if cidx % 2 == 0 else 7)))
        cols = []    # [D, 128] per unit
        bcols = []   # [128] per unit
        for slot in range(NQ):
            if slot < len(qh):
                cols.append(wq[:, qh[slot], :])
                bcols.append(q_bias[qh[slot]])
            else:
                cols.append(np.zeros((D, H), np.float32))
                bcols.append(np.zeros(H, np.float32))
        cols += [wk[:, kv, :], wv[:, kv, :]]
        bcols += [k_bias[kv], v_bias[kv]]
        wqk = np.concatenate(cols, axis=1).reshape(DC, 128, NU * 128)
        biasT = np.stack(bcols, axis=1)                     # [128, NU]
        wo_rows = [wo[qh[sl]] if sl < len(qh) else np.zeros((H, D), np.float32)
                   for sl in range(NQ)]
        woT = np.stack(wo_rows, axis=1).astype(ml_dtypes.bfloat16)  # [128,NQ,D]
        in_maps.append({
            "xT": xT,
            "wqk": np.ascontiguousarray(wqk).astype(ml_dtypes.bfloat16),
            "biasT": biasT, "cosT": cosT, "sinT": sinT,
            "triu": triu, "ones": ones128,
            "woT": np.ascontiguousarray(woT),
        })

    nc = build_program(sched)
    res = run_bass_kernel_spmd(nc, in_maps, list(range(8)), trace=_TRACE)
    if _TRACE and res.exec_time_ns is not None:
        print(f"HW exec time: {res.exec_time_ns} ns")
    y = np.zeros((D, T), np.float64)
    for r in res.results:
        y += r["yT"].astype(np.float64)
    return np.ascontiguousarray(y.T).reshape(B, T, D).astype(np.float32)


# revision 43
# speedup vs baseline: 1.2046x; 1.2046x over previous
"""Trainium2 Bass kernel for GQA attention (B=1,T=2048,D=3584,N=28,KH=4,H=128).

Sharding: 8 cores, one KV head per core-pair. Core c handles kv head c//2 and
query heads [7*(c//2)+4*(c%2) : ...] (4 on even cores, 3+1 dummy on odd cores,
padded to 4 so all cores run one SPMD program).

Per-core dataflow (all "T"-suffixed tensors are feature-major, i.e. transposed):
  stage 1: xT[D,T] (bf16) @ wqk (bf16) -> 6 PSUM units [128,512] per t-stripe;
    each unit drains through an ACT Identity+bias copy (frees the PSUM bank
    fast), then RoPE (plain mul/add) on DVE -> qkT/kT [H,T] f32. The last
    stripe's RoPE is deferred into stage 2 (only window 3 reads it) so the
    DVE queue is empty at the stage boundary. v: bias via ACT -> vT bf16 ->
    v_nat via DMA transpose on the sync queue (no PE, no ACT occupancy).
  stage 2 (causal suffix tiles): per (head, 512-wide t-window) loop over key
    chunks sc: scoresT[s, toff..] = kT_chunk^T . qT_suffix (fp32r, one
    matmul per chunk covering all valid t of the window) -> exp on ACT
    (bf16) -> triangular mask on the 128 diagonal cols (DVE) -> denominator
    accumulated in SBUF (DVE adds) + one ones-matmul per (head,window) ->
    attnV accumulates out_ps[128,512] over chunks -> normalize with DVE
    reciprocal*mul -> attnT bf16. Windows processed in order [1,2,3,0].
  stage 3 (transposed): yT[d_block,t] = sum_u woT_u_block^T . attnT_u
    (bf16) -> bf16 partial yT, summed + transposed on host. Stage-3 groups
    of each finished window are interleaved into the next window's score/
    attnV stream as PE filler, so exp (ACT) latency never starves the PE.
"""

import numpy as np
import ml_dtypes
from contextlib import ExitStack

import concourse.bass as bass
import concourse.bacc as bacc
import concourse.tile as tile
from concourse import mybir
from concourse.bass_utils import run_bass_kernel_spmd

F32 = mybir.dt.float32
F32R = mybir.dt.float32r
BF16 = mybir.dt.bfloat16

B, T, D = 1, 2048, 3584
N, KH, H = 28, 4, 128
G = N // KH              # 7 query heads per kv head
NQ = 4                   # padded query heads per core
NU = NQ + 2              # q0..q3, k, v projection units
DC = D // 128            # 28 contraction chunks
TS = 512                 # stage-1 moving-dim tile
NT = T // TS             # 4
SCC = T // 128           # 16 key chunks
W = 512                  # stage-2/3 t-window (matmul moving dim <= 512)
NWIN = T // W            # 4 windows
SCALE = float(H) ** -0.5
ADD = mybir.AluOpType.add
MUL = mybir.AluOpType.mult

_TRACE = False           # test.py flips this to get an NTFF profile


def build_program(sched):
    """sched: per t-half list of (sc, toff, L, mode); mode 0=clean, 1=diag."""
    nc = bacc.Bacc(None)
    _build_body(nc, sched)
    nc.compile()
    return nc


def _build_body(nc, sched):
    xT_d = nc.dram_tensor("xT", [D, T], BF16, kind="ExternalInput")
    wqk_d = nc.dram_tensor("wqk", [DC, 128, NU * 128], BF16, kind="ExternalInput")
    bias_d = nc.dram_tensor("biasT", [128, NU], F32, kind="ExternalInput")
    cos_d = nc.dram_tensor("cosT", [128, T], F32, kind="ExternalInput")
    sin_d = nc.dram_tensor("sinT", [128, T], F32, kind="ExternalInput")
    triu_d = nc.dram_tensor("triu", [128, 128], BF16, kind="ExternalInput")
    ones_d = nc.dram_tensor("ones", [128, 128], BF16, kind="ExternalInput")
    wo_d = nc.dram_tensor("woT", [128, NQ, D], BF16, kind="ExternalInput")
    yT_d = nc.dram_tensor("yT", [D, T], BF16, kind="ExternalOutput")

    with tile.TileContext(nc) as tc, ExitStack() as ctx:
        persist = ctx.enter_context(tc.tile_pool(name="persist", bufs=1))

        qkT = [persist.tile([128, T], F32R, tag=f"qkT{u}", name=f"qkT{u}")
               for u in range(NQ + 1)]
        # biased (pre-RoPE) last-stripe projections; their RoPE is deferred
        # into stage 2 (results only needed by window 3) so the DVE queue
        # is clear at the stage-1/2 boundary.
        tail_tmp = [persist.tile([128, TS], F32, tag=f"tail{u}",
                                 name=f"tail{u}")
                    for u in range(NQ + 1)]
        v_nat = persist.tile([128, SCC, 128], BF16, tag="vnat")
        attnT = [persist.tile([128, T], BF16, tag=f"attnT{u}", name=f"attnT{u}")
                 for u in range(NQ)]
        cos_sb = persist.tile([128, T], F32, tag="cos")
        sin_sb = persist.tile([128, T], F32, tag="sin")
        bias_sb = persist.tile([128, NU], F32, tag="bias")
        triu_sb = persist.tile([128, 128], BF16, tag="triu")
        ones_sb = persist.tile([128, 128], BF16, tag="ones")

        # ---------------- stage 1: QKV projections + RoPE ----------------
        with tc.tile_pool(name="s1", bufs=1) as wpool, \
             tc.tile_pool(name="s1x", bufs=12) as xpool, \
             tc.tile_pool(name="s1s", bufs=2) as spool, \
             tc.tile_pool(name="s1t", bufs=2) as tpool, \
             tc.tile_pool(name="s1v", bufs=1) as vpool, \
             tc.tile_pool(name="s1ps", bufs=1, space="PSUM") as ps1:

            wqk_sb = [wpool.tile([128, NU * 128], BF16, tag=f"wqk{dc}",
                                 name=f"wqk{dc}") for dc in range(DC)]
            vT_sb = vpool.tile([128, T], BF16)

            for ts in range(NT):
                tsl = slice(ts * TS, (ts + 1) * TS)
                ps = [ps1.tile([128, TS], F32, tag=f"u{u}", name=f"ps_u{u}")
                      for u in range(NU)]
                def epilogue(u):
                    # Drain PSUM via a fast ACT copy that folds the bias
                    # (out = ps + bias) so the next stripe's matmuls (and
                    # stage 2) aren't gated on the slow DVE RoPE chain
                    # reading PSUM. Last stripe's q/k RoPE is deferred
                    # into stage 2 (only window 3 needs it).
                    if u == NU - 1:
                        # v: bias only, un-rotated, bf16; natural layout
                        # via DMA transpose on the sync queue (transpose
                        # triggers occupy their HWDGE engine >1us each and
                        # would delay stage-2 exps if put on ACT).
                        nc.scalar.activation(
                            vT_sb[:, tsl], ps[u][:],
                            mybir.ActivationFunctionType.Identity,
                            bias=bias_sb[:, u:u + 1])
                        for sc in range(ts * (TS // 128),
                                        (ts + 1) * (TS // 128)):
                            nc.sync.dma_start(
                                v_nat[:, sc, :],
                                vT_sb[:, sc * 128:(sc + 1) * 128],
                                transpose=True)
                        return
                    if ts == NT - 1:
                        nc.scalar.activation(
                            tail_tmp[u][:], ps[u][:],
                            mybir.ActivationFunctionType.Identity,
                            bias=bias_sb[:, u:u + 1])
                        return
                    tmp = tpool.tile([128, TS], F32, tag="s1tmp", bufs=5)
                    nc.scalar.activation(
                        tmp[:], ps[u][:],
                        mybir.ActivationFunctionType.Identity,
                        bias=bias_sb[:, u:u + 1])
                    src = tmp
                    dst = qkT[u][:, tsl]
                    c2, s2 = cos_sb[:, tsl], sin_sb[:, tsl]
                    scrA = spool.tile([128, TS], F32, tag="ropeA")
                    scrB = spool.tile([128, TS], F32, tag="ropeB")
                    nc.vector.tensor_mul(scrA[0:64, :], src[64:128, :],
                                         s2[64:128, :])
                    nc.vector.tensor_mul(scrA[64:128, :], src[0:64, :],
                                         s2[0:64, :])
                    nc.vector.tensor_mul(scrB[:], src[:], c2)
                    nc.vector.tensor_add(dst, scrB[:], scrA[:])

                for dc in range(DC):
                    if ts == 0:
                        # interleave weight loads with the first activation
                        # stripe so the PE starts early; misc loads go on
                        # the scalar DMA queue so they never delay x chunks.
                        # weights split across the scalar HWDGE and gpsimd
                        # SWDGE queues: one SWDGE stream alone generates at
                        # ~1.04us/chunk while the PE consumes a chunk every
                        # ~1.3us — no margin. Two queues give 2x headroom.
                        weng = nc.scalar if (dc < 2 or dc % 2) else nc.gpsimd
                        weng.dma_start(wqk_sb[dc][:], wqk_d[dc])
                        if dc == 2:
                            nc.scalar.dma_start(bias_sb[:], bias_d[:])
                            nc.scalar.dma_start(triu_sb[:], triu_d[:])
                            nc.scalar.dma_start(ones_sb[:], ones_d[:])
                        elif dc == 16:
                            # 2MB of rope tables deferred past the startup
                            # DMA crunch (first use is the stripe epilogue)
                            nc.scalar.dma_start(cos_sb[:], cos_d[:])
                            nc.scalar.dma_start(sin_sb[:], sin_d[:])
                    xt = xpool.tile([128, TS], BF16)
                    nc.sync.dma_start(
                        xt[:], xT_d[dc * 128:(dc + 1) * 128, tsl])
                    for u in range(NU):
                        nc.tensor.matmul(
                            ps[u][:],
                            wqk_sb[dc][:, u * 128:(u + 1) * 128],
                            xt[:],
                            start=(dc == 0), stop=(dc == DC - 1))
                        if dc == DC - 1:
                            # emit each unit's drain right after its last
                            # matmul so ACT overlaps the remaining units
                            epilogue(u)

        # ---------------- stage 2 + 3 ----------------
        wopool = ctx.enter_context(tc.tile_pool(name="s3w", bufs=1))
        wo_sb = wopool.tile([128, NQ, D], BF16)
        for u in range(NQ):
            nc.sync.dma_start(wo_sb[:, u, :], wo_d[:, u, :])

        kT = qkT[NQ]

        with tc.tile_pool(name="s2pt", bufs=4) as ptpool, \
             tc.tile_pool(name="s2acc", bufs=2) as accpool, \
             tc.tile_pool(name="s2r", bufs=2) as rpool, \
             tc.tile_pool(name="s3y", bufs=4) as ysbpool, \
             tc.tile_pool(name="s2ps", bufs=1, space="PSUM") as ps2, \
             tc.tile_pool(name="s3ps", bufs=1, space="PSUM") as ps3:

            ncopy = [0]

            def emit_s3_group(db, qt):
                dsl = slice(db * 128, (db + 1) * 128)
                wsl = slice(qt * W, (qt + 1) * W)
                y_ps = ps3.tile([128, W], F32, tag="yT", bufs=3)
                for u in range(NQ):
                    nc.tensor.matmul(
                        y_ps[:], wo_sb[:, u, dsl], attnT[u][:, wsl],
                        start=(u == 0), stop=(u == NQ - 1))
                y_sb = ysbpool.tile([128, W], BF16, tag="ysb")
                # GPSIMD cannot read PSUM; alternate DVE/ACT for the drain
                if ncopy[0] % 2 == 0:
                    nc.vector.tensor_copy(y_sb[:], y_ps[:])
                else:
                    nc.scalar.copy(y_sb[:], y_ps[:])
                ncopy[0] += 1
                nc.sync.dma_start(yT_d[dsl, wsl], y_sb[:])

            def attn_head_window(hq, qt, filler):
                tiles = sched[qt]
                n = len(tiles)
                wsl = slice(qt * W, (qt + 1) * W)
                out_ps = ps2.tile([128, W], F32, tag="out", bufs=2)
                acc = accpool.tile([128, W], BF16, tag="acc")
                pend = None

                def consume(i, sc_ps):
                    sc, toff, L, mode = tiles[i]
                    pt = ptpool.tile([128, W], BF16, tag="pt")
                    nc.scalar.activation(
                        pt[:, 0:L], sc_ps[:, 0:L],
                        mybir.ActivationFunctionType.Exp, scale=SCALE)
                    if mode == 1:
                        nc.vector.tensor_mul(
                            pt[:, 0:128], pt[:, 0:128], triu_sb[:])
                    if i == 0:
                        nc.vector.tensor_copy(acc[:], pt[:])
                    else:
                        nc.vector.tensor_add(
                            acc[:, toff:W], acc[:, toff:W], pt[:, 0:L])
                    nc.tensor.matmul(
                        out_ps[:, toff:W], v_nat[:, sc, :], pt[:, 0:L],
                        start=(i == 0), stop=(i == n - 1),
                        skip_group_check=True)

                for i, (sc, toff, L, mode) in enumerate(tiles):
                    tstart = qt * W + toff
                    sc_ps = ps2.tile([128, W], F32, tag="sc", bufs=3)
                    nc.tensor.matmul(
                        sc_ps[:, 0:L],
                        kT[:, sc * 128:(sc + 1) * 128],
                        qkT[hq][:, tstart:(qt + 1) * W],
                        start=True, stop=True)
                    if filler:
                        filler(i)
                    if pend is not None:
                        consume(*pend)
                    pend = (i, sc_ps)
                consume(*pend)

                den_ps = ps2.tile([128, W], F32, tag="sc", bufs=3)
                nc.tensor.matmul(den_ps[:], ones_sb[:], acc[:],
                                 start=True, stop=True)
                recip = rpool.tile([128, W], F32, tag="recip")
                nc.vector.reciprocal_approx_fast(recip[:], den_ps[:])
                nc.vector.tensor_mul(attnT[hq][:, wsl], out_ps[:], recip[:])

            def emit_tail_rope(u):
                # deferred last-stripe RoPE (needed only by window 3)
                tsl = slice(T - TS, T)
                src = tail_tmp[u]
                dst = qkT[u][:, tsl]
                c2, s2 = cos_sb[:, tsl], sin_sb[:, tsl]
                scrA = rpool.tile([128, TS], F32, tag="dropeA")
                scrB = rpool.tile([128, TS], F32, tag="dropeB")
                nc.vector.tensor_mul(scrA[0:64, :], src[64:128, :],
                                     s2[64:128, :])
                nc.vector.tensor_mul(scrA[64:128, :], src[0:64, :],
                                     s2[0:64, :])
                nc.vector.tensor_mul(scrB[:], src[:], c2)
                nc.vector.tensor_add(dst, scrB[:], scrA[:])

            # Window order [1,2,3,0]: stage-3 groups of the previously
            # finished window interleave as PE filler between the score/
            # attnV matmuls, so exp latency (ACT) never starves the PE.
            # Window 0 (smallest, most latency-bound) runs last, with
            # window-3 fillers available. The deferred last-stripe RoPE
            # (kT after the first window, q units during the second) lands
            # well before window 3 needs it.
            s3q = []
            for wi, qt in enumerate([1, 2, 3, 0]):
                nslots = NQ * len(sched[qt])
                stride = max(1, nslots // 28)

                def filler(i, stride=stride):
                    if s3q and i % stride == 0:
                        emit_s3_group(*s3q.pop(0))

                for hq in range(NQ):
                    attn_head_window(hq, qt, filler)
                    if wi == 1:
                        emit_tail_rope(hq)  # one q unit per head slot
                if wi == 0:
                    emit_tail_rope(NQ)      # kT first
                s3q += [(db, qt) for db in range(DC)]

            # drain last window's stage-3 groups
            while s3q:
                emit_s3_group(*s3q.pop(0))


def _suffix_sched(m):
    """m: bool [T, S]. Per t-half, suffix tiles (sc, toff, L, mode)."""
    tri = np.tril(np.ones((128, 128), bool))
    scheds = []
    for hf in range(NWIN):
        t0 = hf * W
        tiles = []
        for sc in range(SCC):
            sub = m[t0:t0 + W, sc * 128:(sc + 1) * 128]
            rows = sub.any(axis=1)
            if not rows.any():
                continue
            tstart = int(np.argmax(rows))
            if not rows[tstart:].all():
                raise ValueError("mask is not a row-suffix per chunk")
            L = W - tstart
            region = sub[tstart:, :]
            if region.all():
                mode = 0
            elif (L >= 128 and (region[:128] == tri).all()
                  and region[128:].all()):
                mode = 1
            else:
                raise ValueError("unsupported mask pattern")
            tiles.append((sc, tstart, L, mode))
        if not tiles or tiles[0][1] != 0:
            raise ValueError("first chunk must cover the full window")
        scheds.append(tiles)
    return scheds


def kernel(x, attn_mask, sin, cos, wq, wk, wv, wo, q_bias, k_bias, v_bias):
    x = np.asarray(x, np.float32)
    mask = np.asarray(attn_mask).astype(bool)
    sin = np.asarray(sin, np.float32)
    cos = np.asarray(cos, np.float32)
    wq = np.asarray(wq, np.float32)
    wk = np.asarray(wk, np.float32)
    wv = np.asarray(wv, np.float32)
    wo = np.asarray(wo, np.float32)
    q_bias = np.asarray(q_bias, np.float32).reshape(N, H)
    k_bias = np.asarray(k_bias, np.float32).reshape(KH, H)
    v_bias = np.asarray(v_bias, np.float32).reshape(KH, H)

    sched = _suffix_sched(mask[0])

    xT = np.ascontiguousarray(x[0].T).astype(ml_dtypes.bfloat16)   # [D, T]
    c = cos[0].T                                            # [64, T]
    s = sin[0].T
    cosT = np.ascontiguousarray(np.concatenate([c, c], 0))  # [128, T]
    sinT = np.ascontiguousarray(np.concatenate([s, -s], 0))
    triu = np.triu(np.ones((128, 128), np.float32)).astype(ml_dtypes.bfloat16)
    ones128 = np.ones((128, 128), ml_dtypes.bfloat16)

    in_maps = []
    for cidx in range(8):
        kv = cidx // 2
        qh = list(range(7 * kv + 4 * (cidx % 2),
                        7 * kv + (4 if cidx % 2 == 0 else 7)))
        cols = []    # [D, 128] per unit
        bcols = []   # [128] per unit
        for slot in range(NQ):
            if slot < len(qh):
                cols.append(wq[:, qh[slot], :])
                bcols.append(q_bias[qh[slot]])
            else:
                cols.append(np.zeros((D, H), np.float32))
                bcols.append(np.zeros(H, np.float32))
        cols += [wk[:, kv, :], wv[:, kv, :]]
        bcols += [k_bias[kv], v_bias[kv]]
        wqk = np.concatenate(cols, axis=1).reshape(DC, 128, NU * 128)
        biasT = np.stack(bcols, axis=1)                     # [128, NU]
        wo_rows = [wo[qh[sl]] if sl < len(qh) else np.zeros((H, D), np.float32)
                   for sl in range(NQ)]
        woT = np.stack(wo_rows, axis=1).astype(ml_dtypes.bfloat16)  # [128,NQ,D]
        in_maps.append({
            "xT": xT,
            "wqk": np.ascontiguousarray(wqk).astype(ml_dtypes.bfloat16),
            "biasT": biasT, "cosT": cosT, "sinT": sinT,
            "triu": triu, "ones": ones128,
            "woT": np.ascontiguousarray(woT),
        })

    nc = build_program(sched)
    res = run_bass_kernel_spmd(nc, in_maps, list(range(8)), trace=_TRACE)
    if _TRACE and res.exec_time_ns is not None:
        print(f"HW exec time: {res.exec_time_ns} ns")
    y = np.zeros((D, T), np.float64)
    for r in res.results:
        y += r["yT"].astype(np.float64)
    return np.ascontiguousarray(y.T).reshape(B, T, D).astype(np.float32)
